# revision 29
# baseline (speedup 1.0000x reference)
"""Trainium2 Bass kernel for nn_Agent_214748364878 (sparse_attention), v3.

Pure data parallel over batch B=64 -> 8 batches per core. Uses the algebraic
identity  Q . (Kstat + ndf @ Wk) = Q . Kstat + (QWk) . ndf  so the huge
[H,B,M,N,KS] tensors of the reference are never materialized; every big
tensor streams through the PE exactly once.

v3 vs v2 (HW-measured via K-repeat differencing, ~75us -> ~35us/iter):
- Stage B fused: per (batch, chunk) ONE matmul with stationary=ET computes
  U1/S/Z together against an interleaved [vst|ndfn|ones] blob region
  (48 -> 16 matmuls per pair), and the attention tail contracts over d once
  via concat^T = utm2^T @ msel (32 per-head fq/FWl matmuls -> 3).
- Stage C logits matmuls alternate which operand opens each PSUM region
  (S,D | D,S | ...) — halves PE stationary-geometry flips while keeping
  exactly one accumulation group open per bank (two+ open groups corrupt
  the first region; 16 open groups crash the device).
- Five-stage software pipeline (A, Bmm, Btail, Cmm, Ctail) emitted at
  staggered ticks so each in-order engine queue sees work in
  data-readiness order (cuts cross-engine head-of-line blocking).
- _build_program(repeat=K) emits the whole body K times (same output)
  so test.py can measure pure on-device time by slope.

Shapes: B=64, M=5 vehicles, N=1000 nodes, D=128, H=8 heads, KS=16.
Output: softmax probs [64, 5000] (joint softmax over M*N per batch).
"""

import math
import numpy as np

B, M, N, D, H = 64, 5, 1000, 128, 8
KS = D // H
NCORES = 8
BL = B // NCORES          # 8 batches per core
NPAD = 1024
NCHUNK = 8
MF = M * 8                # 40 (m, feature) pairs
HM = H * M                # 40 (head, vehicle) pairs
BM = BL * M               # 40 (batch, vehicle) pairs
BHM = BL * HM             # 320
MASKVAL = -60000.0        # fits fp16; exp underflows to exactly 0

# blob free-dim column offsets (fp16 elements)
O_KST = 0                 # [128=(h,k), 1024=n]
O_LKST = 1024             # [128=d, 1024=n]
O_VN = 2048               # [128=n%128, (c, 169=[vst 128 | ndfn 40 | ones 1])]
VN1 = 169                 # cols per chunk of the fused vst/ndfn/ones region
O_MASKT = 3400            # [128=n%128, (c, 5=m)]  logits mask, 0/-60000
O_MASK01 = 3440           # [128=n%128, (c, 5=m)]  attention mask, 0/1
BLOBC = 3480

# cpack16 column offsets (fp16); prep-critical regions first so the
# first (split) DMA unblocks the prep chain early
C_R8 = 0                  # [8, 40]
C_MASKMA = 40             # [40, 320] (mf, (b,h,m)) same-m
C_BLKM = 360              # [128, 40] ((h',k), (h,m)) same-h
C_WK8 = 400               # [128, 8] W_pns[128:256]
C16A = 408                # prep/rest boundary
C_MASKB5 = 448            # [40, 5]   (mf, m') same-m
C_WVSTK = 453             # [40, 128] (mf, d) = Wv.T tiled
C_POT = 581               # [128, 128] po.T
C_WL8 = 709               # [128, 8] W_pns[256:384]
C_PO = 717                # [128, 128] po (for on-device po.T @ Wl)
C_I40 = 846               # [40, 40] identity (PE transpose)
C_BLKMT = 886             # [40, 128] (h of hm == h of d)
C_MSEL = 1014             # [40, 5]  (m of hm == m')
C_MASKMT2 = 1019          # [40, 40] (m of hm == m of mf)
C16 = 1059

# cpack32 column offsets (fp32); prep-critical regions first
F_WPCVA = 0               # [128, 128] W_pcv[:, :128].T
F_WPCVB = 128             # [3, 128]   W_pcv[:, 128:].T
F_PREVT = 256             # [128, 40]  prev_node_embeddings.T
F_VEHT = 296              # [3, 40]    vehicle_dynamic_features.T
F_FC8 = 336               # [8, 128]   fixed_context rows per batch
F_BSEL2 = 464             # [8, 40]    batch selector [j==b]
F32A = 504                # prep/rest boundary
F_SELT = 504              # [40, 128]  ((h,m), (h',k)) same-h
F_ONEM = 632              # [128, 128] all-ones (partition-sum broadcast)
F_SAMEH = 760             # [40, 40] (h of hm == h of hm')
F32C = 800

_CACHE = {}


def _build_program(repeat=1):
    import concourse.bass as bass
    import concourse.bacc as bacc
    import concourse.tile as tile
    from concourse import mybir

    f32 = mybir.dt.float32
    f16 = mybir.dt.float16
    nc = bacc.Bacc("TRN2", target_bir_lowering=False, debug=False)

    d_blob = nc.dram_tensor("blob", [BL, 128, BLOBC], f16, kind="ExternalInput")
    d_ndftp = nc.dram_tensor("ndftp", [BL // 2, 128, NPAD], f16,
                             kind="ExternalInput")
    d_cp16 = nc.dram_tensor("cp16", [128, C16], f16, kind="ExternalInput")
    d_cp32 = nc.dram_tensor("cp32", [128, F32C], f32, kind="ExternalInput")
    d_out = nc.dram_tensor("out", [128, BL * NCHUNK * M], f32,
                           kind="ExternalOutput")

    mult = mybir.AluOpType.mult
    add = mybir.AluOpType.add
    EXP = mybir.ActivationFunctionType.Exp
    TANH = mybir.ActivationFunctionType.Tanh

    import os as _os
    _pb = lambda k, d: int(_os.environ.get(k, str(d)))
    with tile.TileContext(nc) as tc:
        with (
            tc.tile_pool(name="consts", bufs=min(repeat, 2)) as consts,
            tc.tile_pool(name="persist", bufs=min(repeat, 2)) as persist,
            tc.tile_pool(name="dmab", bufs=_pb("KV2_DMAB", 10)) as dmab,
            tc.tile_pool(name="dman", bufs=_pb("KV2_DMAN", 4)) as dman,
            tc.tile_pool(name="work", bufs=_pb("KV2_WORK", 4)) as work,
            tc.tile_pool(name="ps_ct", bufs=_pb("KV2_PCT", 2),
                         space="PSUM") as ps_ct_pool,
            tc.tile_pool(name="ps_main", bufs=_pb("KV2_PMAIN", 3),
                         space="PSUM") as ps_main_pool,
            tc.tile_pool(name="ps_tail", bufs=_pb("KV2_PTAIL", 3),
                         space="PSUM") as ps_tail_pool,
        ):
          for _rep in range(repeat):
            cp32 = consts.tile([128, F32C], f32, tag="cp32")
            cp16 = consts.tile([128, C16], f16, tag="cp16")
            nc.scalar.dma_start(cp32[:], d_cp32.ap())
            nc.scalar.dma_start(cp16[:], d_cp16.ap())

            # ---------------- prep phase (once, all 8 batches) ----------------
            ps_q = ps_ct_pool.tile([128, BM], f32, tag="ct")
            nc.tensor.matmul(ps_q[:], cp32[:, F_WPCVA:F_WPCVA + 128],
                             cp32[:, F_PREVT:F_PREVT + BM],
                             start=True, stop=False)
            nc.tensor.matmul(ps_q[:], cp32[0:3, F_WPCVB:F_WPCVB + 128],
                             cp32[0:3, F_VEHT:F_VEHT + BM],
                             start=False, stop=False)
            nc.tensor.matmul(ps_q[:], cp32[0:8, F_FC8:F_FC8 + 128],
                             cp32[0:8, F_BSEL2:F_BSEL2 + BM],
                             start=False, stop=True)
            qT_all = work.tile([128, BM], f32, tag="qT_all")
            nc.vector.tensor_copy(qT_all[:], ps_q[:])

            # block-diag Q, all batches: [128=(h,k), (b,h,m)]
            lhsT1 = persist.tile([128, BHM], f16, tag="lhsT1")
            nc.vector.tensor_tensor(
                lhsT1[:].rearrange("p (b h m) -> p b h m", b=BL, h=H),
                qT_all[:].rearrange("p (b m) -> p b m", b=BL)[:, :, None, :]
                .broadcast_to([128, BL, H, M]),
                cp16[:, C_BLKM:C_BLKM + HM]
                .rearrange("p (h m) -> p h m", h=H)[:, None, :, :]
                .broadcast_to([128, BL, H, M]),
                op=mult)

            # per-head QWk replicated over m' -> lhsT2 rows 0:40; mask-bias
            # selector rows 40:45
            ps_qwk = ps_ct_pool.tile([8, BHM], f32, tag="ct")
            nc.tensor.matmul(ps_qwk[:], cp16[:, C_WK8:C_WK8 + 8], lhsT1[:])
            qwk_sb = work.tile([8, BHM], f16, tag="qwk_sb")
            nc.vector.tensor_copy(qwk_sb[:], ps_qwk[:])
            ps_rep = ps_ct_pool.tile([MF, BHM], f32, tag="ct")
            nc.tensor.matmul(ps_rep[:], cp16[0:8, C_R8:C_R8 + MF], qwk_sb[:])
            # duplicated at partition bases 0 and 64 to pair with the
            # two-batches-per-tile ndft layout (matmul requires equal
            # base partitions for lhsT and rhs)
            lhsT2 = persist.tile([64 + MF, BHM], f16, tag="lhsT2")
            for nb in (0, 64):
                nc.vector.tensor_tensor(lhsT2[nb:nb + MF, :], ps_rep[:],
                                        cp16[0:MF, C_MASKMA:C_MASKMA + BHM],
                                        op=mult)

            # powl = po.T @ Wl (on-device weight fusion for the logits
            # dynamic path; lets FWl come straight from concT, parallel to fq)
            ps_powl = ps_tail_pool.tile([128, 8], f32, tag="tail")
            nc.tensor.matmul(ps_powl[:], cp16[:, C_PO:C_PO + 128],
                             cp16[:, C_WL8:C_WL8 + 8])
            powl = persist.tile([128, 8], f16, tag="powl")
            nc.vector.tensor_copy(powl[:], ps_powl[:])

            # ------------- per-batch pipeline, software-pipelined -------------
            out_all = persist.tile([128, BL * NCHUNK * M], f32, tag="out_all")
            import os as _os2
            SmTd = (persist.tile([128, 2 * M], f16, tag="SmTd")
                    if _os2.environ.get("KV2_BPART") == "mm" else None)
            ndft_tiles = {}
            blobs = {}
            state = {}

            def issue_lkst(b):
                nc.sync.dma_start(blobs[b][:, 1024:2048],
                                  d_blob.ap()[b][:, 1024:2048])

            def stage_a(b):
                """DMA in + compat + exp + feasibility mask."""
                blob = dmab.tile([128, BLOBC], f16, tag="blob")
                blobs[b] = blob
                # split by consumer stage: kst (compat) first, then
                # vst/ndfn/masks (acc + masks); the lkst part (only needed
                # by stage C) is deferred two ticks so every batch's
                # B-critical data lands earlier in the saturated DMA stream.
                # Alternate issue queues to overlap DGE pipelines (batch 7
                # forced to SP: the ACT sequencer is compute-busy late).
                eng = nc.sync if (b % 2 == 0 or b == 7) else nc.scalar
                eng.dma_start(blob[:, 0:1024], d_blob.ap()[b][:, 0:1024])
                eng.dma_start(blob[:, 2048:BLOBC],
                              d_blob.ap()[b][:, 2048:BLOBC])
                if b % 2 == 0:
                    nd = dman.tile([128, NPAD], f16, tag="ndft")
                    nc.sync.dma_start(nd[:], d_ndftp.ap()[b // 2])
                    ndft_tiles[b // 2] = nd
                nb = 64 * (b % 2)
                ndft_t = ndft_tiles[b // 2]
                # one spanning accumulation group over the bank: all static
                # matmuls first (they need only lhsT1, ready early in the
                # fill phase), then all dynamic ones (need lhsT2) -- avoids
                # per-chunk head-of-line blocking on the PE stream
                ps_ct = ps_ct_pool.tile([128, NCHUNK * HM], f32, tag="ct")
                for c in range(NCHUNK):
                    cs = slice(c * HM, (c + 1) * HM)
                    nc.tensor.matmul(ps_ct[:, cs],
                                     blob[:, O_KST + c * 128:O_KST + (c + 1) * 128],
                                     lhsT1[:, b * HM:(b + 1) * HM],
                                     start=(c == 0), stop=False,
                                     skip_group_check=True)
                for c in range(NCHUNK):
                    cs = slice(c * HM, (c + 1) * HM)
                    nc.tensor.matmul(ps_ct[:, cs],
                                     ndft_t[nb:nb + MF, c * 128:(c + 1) * 128],
                                     lhsT2[nb:nb + MF, b * HM:(b + 1) * HM],
                                     start=False, stop=(c == NCHUNK - 1),
                                     skip_group_check=True)
                ETu = work.tile([128, NCHUNK * HM], f16, tag="ETu")
                nc.scalar.activation(ETu[:], ps_ct[:], EXP, scale=0.25)
                # 0/1 feasibility mask post-exp; Z/U/S all consume the masked
                # E so this is exact
                ET = work.tile([128, NCHUNK * HM], f16, tag="ET")
                nc.vector.tensor_tensor(
                    ET[:].rearrange("p (c h m) -> p c h m", c=NCHUNK, h=H),
                    ETu[:].rearrange("p (c h m) -> p c h m", c=NCHUNK, h=H),
                    blob[:, O_MASK01:O_MASK01 + NCHUNK * M]
                    .rearrange("p (c m) -> p c m", c=NCHUNK)[:, :, None, :]
                    .broadcast_to([128, NCHUNK, H, M]),
                    op=mult)
                state[b] = dict(blob=blob, ndft=ndft_t, nb=nb, ET=ET)

            pairs = {}

            def stage_bm(p):
                """S/Z/U fused accumulation matmuls for batch pair (2p, 2p+1).
                Per (batch, chunk) ONE matmul with stationary=ET computes U1
                (cols 0:128), S (cols 128:168) and Z (col 168) against the
                fused [vst|ndfn|ones] blob region."""
                b0, b1 = 2 * p, 2 * p + 1
                st0, st1 = state[b0], state[b1]
                import os as _o2
                _bmm = _o2.environ.get("KV2_BPART") == "mm"
                pms = []
                for i, st in ((0, st0), (1, st1)):
                    blob, ET = st["blob"], st["ET"]
                    pm = ps_main_pool.tile([MF, VN1], f32, tag="main")
                    for c in range(NCHUNK):
                        nc.tensor.matmul(pm[:],
                                         ET[:, c * HM:(c + 1) * HM],
                                         blob[:, O_VN + c * VN1:O_VN + (c + 1) * VN1],
                                         start=(c == 0),
                                         stop=(_bmm and c == NCHUNK - 1),
                                         skip_group_check=True)
                    pms.append(pm)
                pairs[p] = dict(pms=pms, st0=st0, st1=st1)

            def stage_bt(p):
                """Attention tail for the pair: U2, 1/Z, concat^T, fq/FWl."""
                pr = pairs[p]
                st0, st1, pms = pr["st0"], pr["st1"], pr["pms"]
                tailt = ps_tail_pool.tile([128, 206], f32, tag="tail")
                # masked S -> fp16, then PE-transpose to [mf, hm] (per batch)
                Smk = work.tile([MF, 2 * MF], f16, tag="Smk")
                for i in (0, 1):
                    nc.vector.tensor_tensor(
                        Smk[:, MF * i:MF * (i + 1)], pms[i][:, 128:168],
                        cp16[0:MF, C_MASKMT2:C_MASKMT2 + MF], op=mult)
                psml = tailt[:, 114:206]
                for i in (0, 1):
                    nc.tensor.matmul(psml[0:MF, HM * i:HM * (i + 1)],
                                     Smk[:, MF * i:MF * (i + 1)],
                                     cp16[0:MF, C_I40:C_I40 + MF],
                                     start=(i == 0), stop=False,
                                     skip_group_check=True)
                # per-head Z for both batches: Z rows live at (h,m); sum over
                # m via the same-head matrix
                r40 = work.tile([MF, 2], f32, tag="r40")
                for i in (0, 1):
                    nc.vector.tensor_copy(r40[:, i:i + 1], pms[i][:, 168:169])
                nc.tensor.matmul(psml[0:MF, 2 * HM:2 * HM + 2],
                                 cp32[0:MF, F_SAMEH:F_SAMEH + MF], r40[:],
                                 start=False, stop=True, skip_group_check=True)
                zinv = work.tile([MF, 2], f32, tag="zinv")
                nc.vector.reciprocal(zinv[:], psml[0:MF, 2 * HM:2 * HM + 2])
                SmT = work.tile([MF, 2 * HM], f16, tag="SmT")
                nc.vector.tensor_copy(SmT[:], psml[0:MF, 0:2 * HM])
                # U2 accumulated into the U region (cols 0:128) of each bank
                for i in (0, 1):
                    nc.tensor.matmul(pms[i][:, 0:128],
                                     SmT[:, HM * i:HM * (i + 1)],
                                     cp16[0:MF, C_WVSTK:C_WVSTK + 128],
                                     start=False, stop=True,
                                     skip_group_check=True)
                # normalized, head-masked U -> utm2 [40(hm), (i, 128=d)]
                utm2 = work.tile([MF, 256], f16, tag="utm2")
                for i in (0, 1):
                    nc.vector.scalar_tensor_tensor(
                        utm2[:, 128 * i:128 * (i + 1)], pms[i][:, 0:128],
                        zinv[:, i:i + 1],
                        cp16[0:MF, C_BLKMT:C_BLKMT + 128], op0=mult, op1=mult)
                # concat^T [128(d), (i, 5=m)] via head-sum matmul
                psc = psml[:, 2 * HM + 2:2 * HM + 12]
                for i in (0, 1):
                    nc.tensor.matmul(psc[:, M * i:M * (i + 1)],
                                     utm2[:, 128 * i:128 * (i + 1)],
                                     cp16[0:MF, C_MSEL:C_MSEL + M],
                                     start=(i == 0), stop=(i == 1),
                                     skip_group_check=True)
                concT = work.tile([128, 2 * M], f16, tag="concT")
                nc.vector.tensor_copy(concT[:], psc[:])
                # fq / FWl: single matmuls contracting over d for the pair
                nc.tensor.matmul(tailt[:, 80:90], cp16[:, C_POT:C_POT + 128],
                                 concT[:], start=True, stop=False,
                                 skip_group_check=True)
                nc.tensor.matmul(tailt[0:8, 94:104], powl[:], concT[:],
                                 start=False, stop=False,
                                 skip_group_check=True)
                fqT = work.tile([128, 2 * M], f16, tag="fqT")
                nc.scalar.activation(fqT[:], tailt[:, 80:90],
                                     mybir.ActivationFunctionType.Copy)
                fwl = work.tile([8, 2 * M], f16, tag="fwl")
                nc.vector.tensor_copy(fwl[:], tailt[0:8, 94:104])
                nc.tensor.matmul(tailt[0:MF, 104:114],
                                 cp16[0:8, C_R8:C_R8 + MF], fwl[:],
                                 start=False, stop=True, skip_group_check=True)
                lhsT3 = work.tile([64 + MF, 2 * M], f16, tag="lhsT3")
                for i in (0, 1):
                    nb = 64 * i
                    nc.vector.tensor_tensor(
                        lhsT3[nb:nb + MF, 5 * i:5 * (i + 1)],
                        tailt[0:MF, 104 + 5 * i:109 + 5 * i],
                        cp16[0:MF, C_MASKB5:C_MASKB5 + M],
                        op=mult)
                pr["tailt"], pr["fqT"], pr["lhsT3"] = tailt, fqT, lhsT3

            def stage_cm(p):
                """Logits matmuls for the pair."""
                pr = pairs[p]
                st0, st1 = pr["st0"], pr["st1"]
                tailt, fqT, lhsT3 = pr["tailt"], pr["fqT"], pr["lhsT3"]
                # statics/dynamics interleaved in blocks of 2 chunks:
                # halves PE stationary-geometry flips while keeping at most
                # two accumulation groups open per bank
                # One open accumulation group at a time: each region's
                # accumulate (start=False) must immediately follow its own
                # start=True (PSUM tracks a single open region; stop is a
                # sim-only marker). Alternate which operand opens so the PE
                # stationary geometry (128-row lkst vs 40-row ndft) flips
                # every TWO matmuls instead of every matmul: S,D | D,S | ...
                for i, st in ((0, st0), (1, st1)):
                    blob, ndft_t, nb = st["blob"], st["ndft"], st["nb"]
                    for c in range(NCHUNK):
                        cs = slice(40 * i + c * M, 40 * i + (c + 1) * M)
                        smm = (tailt[:, cs],
                               blob[:, O_LKST + c * 128:O_LKST + (c + 1) * 128],
                               fqT[:, 5 * i:5 * (i + 1)])
                        dmm = (tailt[:, cs],
                               ndft_t[nb:nb + MF, c * 128:(c + 1) * 128],
                               lhsT3[nb:nb + MF, 5 * i:5 * (i + 1)])
                        first, second = (smm, dmm) if c % 2 == 0 else (dmm, smm)
                        nc.tensor.matmul(*first, start=True, stop=False,
                                         skip_group_check=True)
                        nc.tensor.matmul(*second, start=False, stop=True,
                                         skip_group_check=True)

            def stage_ct(p):
                """Joint softmax + output for the pair."""
                b0, b1 = 2 * p, 2 * p + 1
                pr = pairs.pop(p)
                st0, st1 = pr["st0"], pr["st1"]
                state.pop(b0), state.pop(b1)
                tailt = pr["tailt"]
                tl = work.tile([128, 2 * NCHUNK * M], f32, tag="tl")
                nc.scalar.activation(tl[:], tailt[:, 0:80], TANH,
                                     scale=1.0 / math.sqrt(D))
                pl = work.tile([128, 2 * NCHUNK * M], f32, tag="pl")
                for i, st in ((0, st0), (1, st1)):
                    nc.vector.scalar_tensor_tensor(
                        pl[:, 40 * i:40 * (i + 1)], tl[:, 40 * i:40 * (i + 1)],
                        10.0, st["blob"][:, O_MASKT:O_MASKT + 40],
                        op0=mult, op1=add)
                eL = work.tile([128, 2 * NCHUNK * M], f32, tag="eL")
                rL = work.tile([128, 2], f32, tag="rL")
                nc.scalar.activation(eL[:, 0:40], pl[:, 0:40], EXP,
                                     accum_out=rL[:, 0:1])
                nc.scalar.activation(eL[:, 40:80], pl[:, 40:80], EXP,
                                     accum_out=rL[:, 1:2])
                nc.tensor.matmul(tailt[:, 92:94],
                                 cp32[:, F_ONEM:F_ONEM + 128], rL[:],
                                 skip_group_check=True)
                zbinv = work.tile([128, 2], f32, tag="zbinv")
                nc.vector.reciprocal(zbinv[:], tailt[:, 92:94])
                for i, b in ((0, b0), (1, b1)):
                    nc.vector.tensor_scalar_mul(
                        out_all[:, b * NCHUNK * M:(b + 1) * NCHUNK * M],
                        eL[:, 40 * i:40 * (i + 1)], zbinv[:, i:i + 1])

            import os
            STAGES = int(os.environ.get("KV2_STAGES", "3"))
            BPART_MM = os.environ.get("KV2_BPART") == "mm"
            CPART_MM = os.environ.get("KV2_CPART") == "mm"
            ORDER = os.environ.get("KV2_ORDER", "a,bm,bt,cm,ct").split(",")
            LKLAG = int(os.environ.get("KV2_LKLAG", "5"))
            _stage_map = _CACHE.setdefault("stage_map", [])

            def _mark(tag, fn, *a):
                i0 = nc.next_id()
                fn(*a)
                _stage_map.append((tag, i0, nc.next_id()))

            NP2 = BL // 2
            for t in range(BL + 8):
                for s in ORDER:
                    if s == "a" and t < BL:
                        _mark(f"A{t}", stage_a, t)
                    if s == "a" and 0 <= t - LKLAG < BL:
                        issue_lkst(t - LKLAG)
                    if (s == "bm" and STAGES >= 2 and t % 2 == 1
                            and 0 <= (t - 1) // 2 < NP2):
                        _mark(f"Bm{(t - 1) // 2}", stage_bm, (t - 1) // 2)
                    if (s == "bt" and STAGES >= 2 and not BPART_MM
                            and t % 2 == 0 and 0 <= (t - 2) // 2 < NP2):
                        _mark(f"Bt{(t - 2) // 2}", stage_bt, (t - 2) // 2)
                    if (s == "cm" and STAGES >= 3 and t % 2 == 0
                            and 0 <= (t - 6) // 2 < NP2):
                        _mark(f"Cm{(t - 6) // 2}", stage_cm, (t - 6) // 2)
                    if (s == "ct" and STAGES >= 3 and not CPART_MM
                            and t % 2 == 1 and 0 <= (t - 7) // 2 < NP2):
                        _mark(f"Ct{(t - 7) // 2}", stage_ct, (t - 7) // 2)
            if (STAGES < 3 or os.environ.get("KV2_CPART") == "mm"
                    or os.environ.get("KV2_BPART") == "mm"):
                nc.vector.memset(out_all[:], 0.0)
            half = BL * NCHUNK * M // 2
            nc.sync.dma_start(d_out.ap()[:, 0:half], out_all[:, 0:half])
            nc.sync.dma_start(d_out.ap()[:, half:], out_all[:, half:])

    nc.compile()
    return nc


def _prep_inputs(inputs):
    """Host-side shard + relayout (numpy moves/casts only)."""
    f16 = np.float16
    f32 = np.float32
    gks = np.asarray(inputs["glimpse_K_static"], f32)   # [H,B,1,N,KS]
    gvs = np.asarray(inputs["glimpse_V_static"], f32)
    lks = np.asarray(inputs["logit_K_static"], f32)     # [B,1,N,D]
    ndf = np.asarray(inputs["node_dynamic_features"], f32)  # [B,M,N,8]
    mask = np.asarray(inputs["feasibility_mask"])       # [B,M,N] bool
    prev = np.asarray(inputs["prev_node_embeddings"], f32)  # [B,M,D]
    veh = np.asarray(inputs["vehicle_dynamic_features"], f32)  # [B,M,3]
    fc = np.asarray(inputs["fixed_context"], f32)       # [B,1,D]
    W_pcv = np.asarray(inputs["W_pcv"], f32)            # [D, D+3]
    W_pns = np.asarray(inputs["W_pns"], f32)            # [3D, 8]
    po = np.asarray(inputs["po_weight"], f32)           # [D, D]

    blob = np.zeros((B, 128, BLOBC), dtype=f16)
    # kst: rows (h,k), cols n
    blob[:, :, O_KST:O_KST + N] = (
        gks[:, :, 0].transpose(1, 0, 3, 2).reshape(B, 128, N))
    # lkst: rows d, cols n
    blob[:, :, O_LKST:O_LKST + N] = lks[:, 0].transpose(0, 2, 1)
    # fused vn region: [n%128, (c, [vst 128 | ndfn 40 | ones 1])]
    vn = np.zeros((B, NPAD, VN1), dtype=f16)
    vn[:, :N, 0:128] = gvs[:, :, 0].transpose(1, 2, 0, 3).reshape(B, N, 128)
    vn[:, :N, 128:168] = ndf.transpose(0, 2, 1, 3).reshape(B, N, MF)
    vn[:, :N, 168] = 1.0
    blob[:, :, O_VN:O_VN + NCHUNK * VN1] = (
        vn.reshape(B, NCHUNK, 128, VN1).transpose(0, 2, 1, 3)
        .reshape(B, 128, NCHUNK * VN1))
    # maskT: [n%128, (c, m)] 0/-60000 add-form for the logits path
    mpad = np.full((B, NPAD, M), MASKVAL, dtype=f16)
    mpad[:, :N, :] = np.where(mask, 0.0, MASKVAL).transpose(0, 2, 1)
    blob[:, :, O_MASKT:O_MASKT + NCHUNK * M] = (
        mpad.reshape(B, NCHUNK, 128, M).transpose(0, 2, 1, 3)
        .reshape(B, 128, NCHUNK * M))
    # mask01: [n%128, (c, m)] 0/1 multiply-form for the attention path
    m01 = np.zeros((B, NPAD, M), dtype=f16)
    m01[:, :N, :] = mask.transpose(0, 2, 1).astype(f16)
    blob[:, :, O_MASK01:O_MASK01 + NCHUNK * M] = (
        m01.reshape(B, NCHUNK, 128, M).transpose(0, 2, 1, 3)
        .reshape(B, 128, NCHUNK * M))

    # ndftp: rows 0:40 (m,f) features; two batches per slice (partition
    # bases 0 and 64)
    ndft1 = np.zeros((B, 128, NPAD), dtype=f16)
    ndft1[:, :MF, :N] = ndf.transpose(0, 1, 3, 2).reshape(B, MF, N)

    # constants
    cp16 = np.zeros((128, C16), dtype=f16)
    r8 = np.zeros((8, MF), dtype=f16)
    for m in range(M):
        for ff in range(8):
            r8[ff, m * 8 + ff] = 1.0
    cp16[0:8, C_R8:C_R8 + MF] = r8
    mf_m = np.arange(MF) // 8                      # m of each (m,f) row
    hm_m = np.arange(HM) % M                       # m of each (h,m) col
    cp16[0:MF, C_MASKB5:C_MASKB5 + M] = (
        mf_m[:, None] == np.arange(M)[None, :]).astype(f16)
    bhm_m = np.arange(BHM) % M                     # m of each (b,h,m) col
    cp16[0:MF, C_MASKMA:C_MASKMA + BHM] = (
        mf_m[:, None] == bhm_m[None, :]).astype(f16)
    cp16[0:MF, C_WVSTK:C_WVSTK + 128] = np.tile(
        W_pns[0:128].T.reshape(1, 8, 128), (M, 1, 1)).reshape(MF, 128)
    d_h = np.arange(128) // KS                     # h of each (h,k) row
    hm_h = np.arange(HM) // M                      # h of each (h,m) col
    cp16[:, C_BLKM:C_BLKM + HM] = (
        d_h[:, None] == hm_h[None, :]).astype(f16)
    cp16[:, C_POT:C_POT + 128] = po.T
    cp16[:, C_WL8:C_WL8 + 8] = W_pns[256:384]
    cp16[:, C_PO:C_PO + 128] = po
    cp16[:, C_WK8:C_WK8 + 8] = W_pns[128:256]
    cp16[0:MF, C_I40:C_I40 + MF] = np.eye(MF, dtype=f16)
    cp16[0:MF, C_BLKMT:C_BLKMT + 128] = (
        hm_h[:, None] == d_h[None, :]).astype(f16)
    cp16[0:MF, C_MSEL:C_MSEL + M] = (
        hm_m[:, None] == np.arange(M)[None, :]).astype(f16)
    cp16[0:MF, C_MASKMT2:C_MASKMT2 + MF] = (
        hm_m[:, None] == mf_m[None, :]).astype(f16)

    cp32 = np.zeros((128, F32C), dtype=f32)
    cp32[:, F_WPCVA:F_WPCVA + 128] = W_pcv[:, 0:128].T
    cp32[0:3, F_WPCVB:F_WPCVB + 128] = W_pcv[:, 128:131].T
    sel = np.zeros((HM, 128), dtype=f32)
    for h in range(H):
        sel[h * M:(h + 1) * M, h * KS:(h + 1) * KS] = 1.0
    cp32[0:HM, F_SELT:F_SELT + 128] = sel
    cp32[:, F_ONEM:F_ONEM + 128] = 1.0
    cp32[0:MF, F_SAMEH:F_SAMEH + MF] = (
        hm_h[:, None] == hm_h[None, :]).astype(f32)

    in_maps = []
    for cid in range(NCORES):
        sl = slice(cid * BL, (cid + 1) * BL)
        c32 = cp32.copy()
        # fc rows per batch + batch selector, prev/veh transposed
        c32[0:BL, F_FC8:F_FC8 + 128] = fc[sl, 0]
        c32[0:BL, F_BSEL2:F_BSEL2 + BM] = (
            np.arange(BL)[:, None] == (np.arange(BM) // M)[None, :])
        c32[:, F_PREVT:F_PREVT + BM] = (
            prev[sl].transpose(2, 0, 1).reshape(128, BM))
        c32[0:3, F_VEHT:F_VEHT + BM] = (
            veh[sl].transpose(2, 0, 1).reshape(3, BM))
        ndp = np.zeros((BL // 2, 128, NPAD), dtype=f16)
        nd = ndft1[sl]
        ndp[:, 0:MF] = nd[0::2, :MF]
        ndp[:, 64:64 + MF] = nd[1::2, :MF]
        in_maps.append(dict(blob=blob[sl], ndftp=ndp,
                            cp16=cp16.copy(), cp32=c32))
    return in_maps


def kernel(**inputs) -> np.ndarray:
    from concourse import bass_utils

    if "nc" not in _CACHE:
        _CACHE["nc"] = _build_program()
    nc = _CACHE["nc"]
    in_maps = _prep_inputs(inputs)
    res = bass_utils.run_bass_kernel_spmd(nc, in_maps, core_ids=list(range(NCORES)))
    outs = []
    for c in range(NCORES):
        o = res.results[c]["out"]                  # [128, (b, c, m)]
        o = (o.reshape(128, BL, NCHUNK, M).transpose(1, 3, 2, 0)
             .reshape(BL, M, NPAD)[:, :, :N].reshape(BL, M * N))
        outs.append(o)
    return np.concatenate(outs, axis=0).astype(np.float32)

